# revision 2
# baseline (speedup 1.0000x reference)
"""Spiking autoencoder (integrate-and-fire, 16 timesteps) on 8 TRN2 NeuronCores.

Data-parallel: batch 16384 split as 8 x 2048. Per core, a fully fused
Bass/Tile kernel:

  - features are PE-transposed to feature-major [896(pad), B] layout and
    quantized (round(16x)/16) with one fused DVE op.
  - Each layer's membrane potential uses the cumulative-potential trick:
    PSUM accumulates V_t = sum_{tau<=t} W @ s_tau across all 16 steps
    (matmul start= only on t==0), and the integrate-and-fire recurrence is
    run against V_t with an integer spike-count state:
        s_t = [h + V_t >= 1],  h -= s_t        (h = -#spikes so far, exact)
    which equals the reference's (p += u; s = [p>=1]; p -= s) chain.
  - Layer 4 needs only the output spike COUNT:  C += [V4 - C >= 1], one
    fused DVE op per step, no s4/p4 materialization.
  - Matmuls run as float32r (full-rate on the PE array).

Custom DVE ops (fused, registered at import): ANT_SPIKE, ANT_FIRE,
ANT_DROP, ANT_COUNT, ANT_ROUND16.
"""
import sys
import copy
import itertools

sys.path.insert(0, "/opt/trn_rl_repo")

import numpy as np

# ----------------------------------------------------------------------------
# Custom DVE op registration
# ----------------------------------------------------------------------------
import concourse.dve_ops as dve_ops
from concourse.dve_ops import DveOp
from concourse.dve_spec import (
    Spec, Src0, Src1, One, Zero, C0, C1, C2, lower, _has_src1 as has_src1,
)
from concourse.dve_table_gen import DveOpSpec

_F = np.float32


def _register(name, spec):
    if name in dve_ops._SUB_OPCODE_FOR_NAME:
        return next(op for op in dve_ops.OPS if op.name == name)
    shas = {}
    for ver in ("v3", "v4"):
        s = DveOpSpec(name=name, opcode=0, uops=lower(spec, ver=ver),
                      rd1_en=has_src1(spec))
        shas[ver] = s.sha(ver)
    op = DveOp(name, spec, subdim=False, uops_sha=shas)
    dve_ops.OPS.append(op)
    dve_ops._SUB_OPCODE_FOR_NAME[name] = (
        dve_ops._CUSTOM_DVE_ROW_BASE + len(dve_ops.OPS) - 1)
    dve_ops.CUSTOM_DVE_SPECS[name] = spec
    assert dve_ops._SUB_OPCODE_FOR_NAME[name] < 0x20
    return op


# s = (p + u) >= 1
ANT_SPIKE = _register("ANT_SPIKE", Spec(
    body=(Src0 + Src1) >= One,
    reference=lambda in0, in1, s0, s1, imm2: ((in0 + in1) >= 1.0).astype(_F)))

# p' = q - (q >= 1), q = p + u   (state update when u is the per-step input)
_q = Src0 + Src1
ANT_FIRE = _register("ANT_FIRE", Spec(
    body=_q - (_q >= One),
    reference=lambda in0, in1, s0, s1, imm2:
        (in0 + in1) - ((in0 + in1) >= 1.0).astype(_F)))

# h' = h - ((h + V) >= 1)   (state update when V is cumulative)
ANT_DROP = _register("ANT_DROP", Spec(
    body=Src0 - ((Src0 + Src1) >= One),
    reference=lambda in0, in1, s0, s1, imm2:
        in0 - ((in0 + in1) >= 1.0).astype(_F)))

# C' = C + ((V - C) >= 1)
ANT_COUNT = _register("ANT_COUNT", Spec(
    body=Src0 + ((Src1 - Src0) >= One),
    reference=lambda in0, in1, s0, s1, imm2:
        in0 + ((in1 - in0) >= 1.0).astype(_F)))

# R_1 = x - (x >= 1)   (first fire step from zero state)
ANT_FIRE0 = _register("ANT_FIRE0", Spec(
    body=Src0 - (Src0 >= One),
    reference=lambda in0, s0, s1, imm2: in0 - (in0 >= 1.0).astype(_F)))

# h' = -(V >= 1)   (first cumulative-drop step from zero state)
ANT_DROP0 = _register("ANT_DROP0", Spec(
    body=Zero - (Src0 >= One),
    reference=lambda in0, s0, s1, imm2: -((in0 >= 1.0).astype(_F))))

# out = ((x*C0 + C1) - C1) * C2  -> round-to-nearest-even via the 2^23 trick
ANT_ROUND16 = _register("ANT_ROUND16", Spec(
    body=((Src0 * C0 + C1) - C1) * C2,
    reference=lambda in0, s0, s1, imm2: (
        (np.float32(in0 * np.float32(s0)) + np.float32(s1))
        - np.float32(s1)) * np.float32(imm2)))

# ----------------------------------------------------------------------------
# Walrus-compat fixes (this container's neuronxcc rejects >1 sem-wait on
# many instruction structs and any wait on InstDrain; raw Bass also skips
# the pass that packs extended-inst ISA bytes).
# ----------------------------------------------------------------------------
from concourse import bass, mybir
from concourse.tile import TileContext
from concourse.vector_clock import ScopedClock
from concourse.bass_utils import run_bass_kernel_spmd
from concourse.masks import make_identity

_ctr = itertools.count()


def _build_wait_templates():
    nc = bass.Bass(target_bir_lowering=False)
    out = {}
    with nc.Block() as block, nc.semaphore("s") as s:
        for eng_name in ("sync", "vector", "scalar", "gpsimd", "tensor"):
            def _mk(e, _out=out):
                i = e.wait_ge(s, 0)
                _out[i.ins.engine] = i.ins
            getattr(block, eng_name)(_mk)
    return out


_WAIT_TEMPLATES = _build_wait_templates()


def _mk_wait(engine, w):
    wi = copy.deepcopy(_WAIT_TEMPLATES[engine])
    wi.name = f"I-waitsplit-{next(_ctr)}"
    si = wi.sync_info
    si.on_wait.clear()
    si.on_wait.append(w)
    return wi


def _fix_waits(nc, limit=1):
    n = 0
    for bb in nc.main_func.blocks:
        il = bb.instructions
        i = 0
        while i < len(il):
            ins = il[i]
            lim = 0 if type(ins).__name__ == "InstDrain" else limit
            si = ins.sync_info
            waits = list(si.on_wait) if (si and si.on_wait) else []
            if type(ins).__name__ != "InstEventSemaphore" and len(waits) > lim:
                keep, extra = waits[:lim], waits[lim:]
                si.on_wait.clear()
                for w in keep:
                    si.on_wait.append(w)
                for j, w in enumerate(extra):
                    il.insert(i + j, _mk_wait(ins.engine, w))
                i += len(extra)
                n += 1
            i += 1
    return n


def _finalize(nc):
    from concourse.library_overlay import lower_extended_insts
    lower_extended_insts(nc)
    return _fix_waits(nc)


def _patched_drain_and_barrier(self, tick_clock, wait_clock):
    nc = self.nc
    probe = nc.sync.nop()
    wait_clock.add_sem_waits(probe.ins, ScopedClock({None: tick_clock.global_clock}))
    si = probe.ins.sync_info
    waits = list(si.on_wait or []) if si is not None else []
    if si is not None and si.on_wait:
        si.on_wait.clear()
    handles = list(self.sems.allocated().values())
    by_name = {getattr(h, "name", None): h for h in handles}
    for w in waits:
        nc.sync.wait_ge(by_name[w.ant_name], w.wait_value)
    nc.sync.drain()
    nc.all_engine_barrier()
    popped = nc._tile_sem_poison_stack.pop()
    assert popped is self._sem_poison
    nc.clear_and_free_semaphores(handles)
    nc.all_engine_barrier()


TileContext._drain_and_barrier = _patched_drain_and_barrier

# ----------------------------------------------------------------------------
# Kernel build
# ----------------------------------------------------------------------------
F32 = mybir.dt.float32
F32R = mybir.dt.float32r

NCORES = 8
B = 16384
BL = B // NCORES          # 2048 per core
IN = 784
H = 128
T = 16
FT = 7                    # feature tiles
F = FT * 128              # 896 padded
BC = 512                  # batch chunk (psum-bank limited)
NCH = BL // BC            # 4 chunks
NBT = BC // 128           # 4 batch subtiles per chunk

_CACHE = {}


def _build():
    if "nc" in _CACHE:
        return _CACHE["nc"]
    nc = bass.Bass(target_bir_lowering=False)
    x_ext = nc.declare_dram_parameter("x", [BL, IN], F32, isOutput=False)
    w_ext = nc.declare_dram_parameter("wts", [128, 4096], F32R, isOutput=False)
    sc_ext = nc.declare_dram_parameter("sc", [1, 1], F32, isOutput=False)
    o_ext = nc.declare_dram_parameter("out", [BL, IN], F32, isOutput=True)

    with TileContext(nc) as tc:
        with (tc.tile_pool(name="const", bufs=1) as constp,
              tc.tile_pool(name="sb", bufs=2) as sb,
              tc.tile_pool(name="st", bufs=1) as st,
              tc.tile_pool(name="st2", bufs=2) as st2,
              tc.tile_pool(name="ps", bufs=1, space="PSUM") as ps):

            wts = constp.tile([128, 4096], F32R, tag="wts")
            ident = constp.tile([128, 128], F32, tag="ident")
            scb = constp.tile([128, 1], F32, tag="scb")
            nc.sync.dma_start(out=wts[:], in_=w_ext[:])
            nc.sync.dma_start(out=scb[:], in_=sc_ext[:].to_broadcast([128, 1]))
            make_identity(nc, ident[:])
            nc.vector.tensor_scalar_mul(scb[:], scb[:], 1.0 / T)

            def _wslices(base):
                return ([wts[:, base + k * 128:base + (k + 1) * 128]
                         for k in range(FT)],
                        wts[:, base + 896:base + 1024],
                        wts[:, base + 1024:base + 1152],
                        [wts[:, base + 1152 + j * 128:base + 1152 + (j + 1) * 128]
                         for j in range(FT)])
            w1s, w2s, w3s, w4s = zip(_wslices(0), _wslices(2048))

            for c in range(NCH):
                inp = sb.tile([128, FT, BC], F32R, tag="inp", name=f"inp_{c}")
                Rb = [st.tile([128, FT, BC], F32R, tag=f"R{i}", name=f"R{i}_{c}")
                      for i in range(2)]
                g1 = st.tile([128, BC], F32, tag="g1", name=f"g1_{c}")
                h2 = st.tile([128, BC], F32, tag="h2", name=f"h2_{c}")
                h3b = [st.tile([128, BC], F32R, tag=f"h3{i}", name=f"h3{i}_{c}")
                       for i in range(2)]
                A1 = st.tile([128, BC], F32, tag="A1", name=f"A1_{c}")
                C = st2.tile([128, FT, BC], F32, tag="C", name=f"C_{c}")

                # ---- input: DMA batch-major, PE-transpose, quantize ----
                for b in range(NBT):
                    xt = sb.tile([128, IN], F32, tag="x", name=f"x_{c}_{b}")
                    nc.sync.dma_start(
                        out=xt[:],
                        in_=x_ext[c * BC + b * 128:c * BC + (b + 1) * 128, :])
                    xps = ps.tile([128, 4, 128], F32, tag="xtps",
                                  name=f"xpsA_{c}_{b}")
                    for j in range(4):
                        nc.tensor.transpose(
                            xps[:, j, :], xt[:, j * 128:(j + 1) * 128],
                            identity=ident[:])
                    nc.vector._custom_dve(
                        ANT_ROUND16,
                        out=inp[:, 0:4, b * 128:(b + 1) * 128],
                        in0=xps[:, :, :],
                        s0=16.0, s1=float(2 ** 23), imm2=1.0 / 16.0)
                    xps2 = ps.tile([128, 3, 128], F32, tag="xtps",
                                   name=f"xpsB_{c}_{b}")
                    for j in range(4, 6):
                        nc.tensor.transpose(
                            xps2[:, j - 4, :], xt[:, j * 128:(j + 1) * 128],
                            identity=ident[:])
                    nc.vector.memset(xps2[:, 2, :], 0.0)
                    nc.tensor.transpose(
                        xps2[0:16, 2, :], xt[:, 768:784], identity=ident[:])
                    nc.vector._custom_dve(
                        ANT_ROUND16,
                        out=inp[:, 4:7, b * 128:(b + 1) * 128],
                        in0=xps2[:, :, :],
                        s0=16.0, s1=float(2 ** 23), imm2=1.0 / 16.0)

                nc.gpsimd.memset(g1[:], 0.0)
                nc.gpsimd.memset(h2[:], 0.0)
                nc.gpsimd.memset(C[:], 0.0)

                # ---- A1 = W1 @ inp (once per chunk) ----
                a1ps = ps.tile([128, BC], F32, tag="xtps", name=f"a1ps_{c}")
                for k in range(FT):
                    for h in range(2):
                        nc.tensor.matmul(a1ps[:], w1s[h][k], inp[:, k, :],
                                         start=(k == 0 and h == 0),
                                         stop=(k == FT - 1 and h == 1))
                # stationary is -W1, so negate to get +A1
                nc.vector.tensor_scalar_mul(A1[:], a1ps[:], -1.0)

                V123 = ps.tile([128, 3, BC], F32, tag="V123", name=f"V123_{c}")
                V1 = V123[:, 0, :]
                V2 = V123[:, 1, :]
                V3 = V123[:, 2, :]

                # ---- 16 steps; layer-4 psum is FRESH per (step, half):
                #      V4 = -W4 @ h3_t  (h3 = -cum spikes, exact), so no s3
                #      materialization and only one 4-bank psum slot for D ----
                HB = BC // 2
                # prologue: R_1 = fire(0 + inp)
                nc.vector._custom_dve(ANT_FIRE0, out=Rb[0][:, 0:4, :], in0=inp[:, 0:4, :])
                nc.vector._custom_dve(ANT_FIRE0, out=Rb[0][:, 4:7, :], in0=inp[:, 4:7, :])

                def d_block(tprev, halves=(0, 1)):
                    """layer 4 for step tprev (runs one iteration later):
                    V4 = -W4 @ h3_tprev, fresh psum per half + COUNT."""
                    h3 = h3b[tprev % 2]
                    for half in halves:
                        lo = half * HB
                        V4h = ps.tile([128, FT, HB], F32, tag="V4h",
                                      name=f"V4h_{c}_{tprev}_{half}")
                        for j in range(FT):
                            for h in range(2):
                                nc.tensor.matmul(
                                    V4h[:, j, :], w4s[h][j], h3[:, lo:lo + HB],
                                    start=(h == 0), stop=(h == 1))
                        nc.vector._custom_dve(
                            ANT_COUNT, out=C[:, :, lo:lo + HB],
                            in0=C[:, :, lo:lo + HB], in1=V4h[:])

                for t in range(T):
                    R = Rb[t % 2]
                    Rn = Rb[(t + 1) % 2]
                    s1 = sb.tile([128, BC], F32R, tag="s1", name=f"s1_{c}_{t}")
                    s2 = sb.tile([128, BC], F32R, tag="s2", name=f"s2_{c}_{t}")
                    # V1_t = t*A1 - W1 @ R_t   (fresh psum each step)
                    for k in range(FT):
                        for h in range(2):
                            nc.tensor.matmul(V1[:], w1s[h][k], R[:, k, :],
                                             start=(k == 0 and h == 0),
                                             stop=(k == FT - 1 and h == 1))
                    # layer-4 work of the PREVIOUS step fills the L2/L3 stretch
                    if t > 0:
                        d_block(t - 1, halves=(0,))
                    if t < T - 1:
                        nc.vector._custom_dve(ANT_FIRE, out=Rn[:, 0:4, :],
                                              in0=R[:, 0:4, :], in1=inp[:, 0:4, :])
                    nc.gpsimd.tensor_add(g1[:], g1[:], A1[:])
                    nc.vector._custom_dve(ANT_SPIKE, out=s1[:], in0=g1[:], in1=V1[:])
                    if t < T - 1:
                        nc.vector._custom_dve(ANT_FIRE, out=Rn[:, 4:7, :],
                                              in0=R[:, 4:7, :], in1=inp[:, 4:7, :])
                    nc.vector._custom_dve(ANT_DROP, out=g1[:], in0=g1[:], in1=V1[:])
                    for h in range(2):
                        nc.tensor.matmul(V2[:], w2s[h], s1[:],
                                         start=(t == 0 and h == 0),
                                         stop=(t == T - 1 and h == 1))
                    nc.vector._custom_dve(ANT_SPIKE, out=s2[:], in0=h2[:], in1=V2[:])
                    nc.vector._custom_dve(ANT_DROP, out=h2[:], in0=h2[:], in1=V2[:])
                    for h in range(2):
                        nc.tensor.matmul(V3[:], w3s[h], s2[:],
                                         start=(t == 0 and h == 0),
                                         stop=(t == T - 1 and h == 1))
                    if t > 0:
                        d_block(t - 1, halves=(1,))
                    if t == 0:
                        nc.vector._custom_dve(ANT_DROP0, out=h3b[0][:], in0=V3[:])
                    else:
                        nc.vector._custom_dve(ANT_DROP, out=h3b[t % 2][:],
                                              in0=h3b[(t - 1) % 2][:], in1=V3[:])
                # epilogue: layer 4 of the final step
                d_block(T - 1)

                # ---- output: transpose back, scale, DMA ----
                for b in range(NBT):
                    cps = ps.tile([128, FT, 128], F32, tag="V4h",
                                  name=f"coT_{c}_{b}")
                    for j in range(FT):
                        nc.tensor.transpose(
                            cps[:, j, :], C[:, j, b * 128:(b + 1) * 128],
                            identity=ident[:])
                    yo = sb.tile([128, FT, 128], F32, tag="yo",
                                 name=f"yo_{c}_{b}")
                    nc.vector.tensor_scalar(
                        out=yo[:], in0=cps[:], scalar1=scb[:], scalar2=None,
                        op0=mybir.AluOpType.mult)
                    orows = slice(c * BC + b * 128, c * BC + (b + 1) * 128)
                    for j in range(6):
                        nc.gpsimd.dma_start(
                            out=o_ext[orows, j * 128:(j + 1) * 128],
                            in_=yo[:, j, :])
                    nc.gpsimd.dma_start(out=o_ext[orows, 768:784],
                                        in_=yo[:, 6, 0:16])

    _finalize(nc)
    _CACHE["nc"] = nc
    return nc


def _rne11(x):
    xi = np.asarray(x, np.float32).view(np.uint32).astype(np.uint64)
    half = np.uint64(1 << 11)
    lsb = (xi >> np.uint64(12)) & np.uint64(1)
    q = ((xi + half - np.uint64(1) + lsb) >> np.uint64(12)) << np.uint64(12)
    return np.minimum(q, np.uint64(0xFFFFFFFF)).astype(np.uint32).view(np.float32)


def _prep_inputs(features, W1, W2, W3, W4, out_scale):
    f32 = np.float32
    W1p = np.zeros((H, F), f32); W1p[:, :IN] = W1
    W4p = np.zeros((F, H), f32); W4p[:IN, :] = W4
    W1T = W1p.T.reshape(FT, 128, H).transpose(1, 0, 2).reshape(128, FT * H)
    whole = np.concatenate(
        [-W1T, W2.T.astype(f32), W3.T.astype(f32), -W4p.T], axis=1)
    hi = _rne11(whole)
    lo = (whole - hi).astype(f32)
    wts = np.ascontiguousarray(np.concatenate([hi, lo], axis=1), dtype=f32)
    sc = np.asarray(out_scale, f32).reshape(1, 1)
    in_maps = []
    for i in range(NCORES):
        in_maps.append({
            "x": np.ascontiguousarray(features[i * BL:(i + 1) * BL], f32),
            "wts": wts,
            "sc": sc,
        })
    return in_maps


def _run(inputs, trace=False):
    nc = _build()
    in_maps = _prep_inputs(**inputs)
    res = run_bass_kernel_spmd(nc, in_maps, core_ids=list(range(NCORES)),
                               trace=trace)
    out = np.concatenate([res.results[i]["out"] for i in range(NCORES)], axis=0)
    return out.astype(np.float32), res


def kernel(**inputs):
    out, _ = _run(inputs, trace=False)
    return out


# revision 3
# speedup vs baseline: 1.3837x; 1.3837x over previous
"""Spiking autoencoder (integrate-and-fire, 16 timesteps) on 8 TRN2 NeuronCores.

Data-parallel: batch 16384 split as 8 x 2048. Per core, a fully fused
Bass/Tile kernel:

  - features are PE-transposed to feature-major [896(pad), B] layout and
    quantized (round(16x)/16) with one fused DVE op.
  - Each layer's membrane potential uses the cumulative-potential trick:
    PSUM accumulates V_t = sum_{tau<=t} W @ s_tau across all 16 steps
    (matmul start= only on t==0), and the integrate-and-fire recurrence is
    run against V_t with an integer spike-count state:
        s_t = [h + V_t >= 1],  h -= s_t        (h = -#spikes so far, exact)
    which equals the reference's (p += u; s = [p>=1]; p -= s) chain.
  - Layers 3/4 exploit that the h-state is an exact integer count: their
    potentials are rebuilt fresh each step as -W @ h (so s2/s3 are never
    materialized), and layer 4 keeps only the output spike COUNT:
    C += [V4 - C >= 1].
  - Matmuls run as float32r (full-rate on the PE array); every weight is
    split hi=rne11(W), lo=W-hi so two accumulating fp32r matmuls reproduce
    fp32 precision.

Custom DVE ops (fused, registered at import): ANT_SPIKE, ANT_FIRE,
ANT_DROP, ANT_COUNT, ANT_ROUND16.
"""
import sys
import copy
import itertools

sys.path.insert(0, "/opt/trn_rl_repo")

import numpy as np

# ----------------------------------------------------------------------------
# Custom DVE op registration
# ----------------------------------------------------------------------------
import concourse.dve_ops as dve_ops
from concourse.dve_ops import DveOp
from concourse.dve_spec import (
    Spec, Src0, Src1, One, Zero, C0, C1, C2, lower, _has_src1 as has_src1,
)
from concourse.dve_table_gen import DveOpSpec

_F = np.float32


def _register(name, spec):
    if name in dve_ops._SUB_OPCODE_FOR_NAME:
        return next(op for op in dve_ops.OPS if op.name == name)
    shas = {}
    for ver in ("v3", "v4"):
        s = DveOpSpec(name=name, opcode=0, uops=lower(spec, ver=ver),
                      rd1_en=has_src1(spec))
        shas[ver] = s.sha(ver)
    op = DveOp(name, spec, subdim=False, uops_sha=shas)
    dve_ops.OPS.append(op)
    dve_ops._SUB_OPCODE_FOR_NAME[name] = (
        dve_ops._CUSTOM_DVE_ROW_BASE + len(dve_ops.OPS) - 1)
    dve_ops.CUSTOM_DVE_SPECS[name] = spec
    assert dve_ops._SUB_OPCODE_FOR_NAME[name] < 0x20
    return op


# s = (p + u) >= 1
ANT_SPIKE = _register("ANT_SPIKE", Spec(
    body=(Src0 + Src1) >= One,
    reference=lambda in0, in1, s0, s1, imm2: ((in0 + in1) >= 1.0).astype(_F)))

# p' = q - (q >= 1), q = p + u   (state update when u is the per-step input)
_q = Src0 + Src1
ANT_FIRE = _register("ANT_FIRE", Spec(
    body=_q - (_q >= One),
    reference=lambda in0, in1, s0, s1, imm2:
        (in0 + in1) - ((in0 + in1) >= 1.0).astype(_F)))

# h' = h - ((h + V) >= 1)   (state update when V is cumulative)
ANT_DROP = _register("ANT_DROP", Spec(
    body=Src0 - ((Src0 + Src1) >= One),
    reference=lambda in0, in1, s0, s1, imm2:
        in0 - ((in0 + in1) >= 1.0).astype(_F)))

# C' = C + ((V - C) >= 1)
ANT_COUNT = _register("ANT_COUNT", Spec(
    body=Src0 + ((Src1 - Src0) >= One),
    reference=lambda in0, in1, s0, s1, imm2:
        in0 + ((in1 - in0) >= 1.0).astype(_F)))

# R_1 = x - (x >= 1)   (first fire step from zero state)
ANT_FIRE0 = _register("ANT_FIRE0", Spec(
    body=Src0 - (Src0 >= One),
    reference=lambda in0, s0, s1, imm2: in0 - (in0 >= 1.0).astype(_F)))

# h' = -(V >= 1)   (first cumulative-drop step from zero state)
ANT_DROP0 = _register("ANT_DROP0", Spec(
    body=Zero - (Src0 >= One),
    reference=lambda in0, s0, s1, imm2: -((in0 >= 1.0).astype(_F))))

# out = ((x*C0 + C1) - C1) * C2  -> round-to-nearest-even via the 2^23 trick
ANT_ROUND16 = _register("ANT_ROUND16", Spec(
    body=((Src0 * C0 + C1) - C1) * C2,
    reference=lambda in0, s0, s1, imm2: (
        (np.float32(in0 * np.float32(s0)) + np.float32(s1))
        - np.float32(s1)) * np.float32(imm2)))

# ----------------------------------------------------------------------------
# Walrus-compat fixes (this container's neuronxcc rejects >1 sem-wait on
# many instruction structs and any wait on InstDrain; raw Bass also skips
# the pass that packs extended-inst ISA bytes).
# ----------------------------------------------------------------------------
from concourse import bass, mybir
from concourse.tile import TileContext
from concourse.vector_clock import ScopedClock
from concourse.bass_utils import run_bass_kernel_spmd
from concourse.masks import make_identity

_ctr = itertools.count()


def _build_wait_templates():
    nc = bass.Bass(target_bir_lowering=False)
    out = {}
    with nc.Block() as block, nc.semaphore("s") as s:
        for eng_name in ("sync", "vector", "scalar", "gpsimd", "tensor"):
            def _mk(e, _out=out):
                i = e.wait_ge(s, 0)
                _out[i.ins.engine] = i.ins
            getattr(block, eng_name)(_mk)
    return out


_WAIT_TEMPLATES = _build_wait_templates()


def _mk_wait(engine, w):
    wi = copy.deepcopy(_WAIT_TEMPLATES[engine])
    wi.name = f"I-waitsplit-{next(_ctr)}"
    si = wi.sync_info
    si.on_wait.clear()
    si.on_wait.append(w)
    return wi


def _fix_waits(nc, limit=1):
    n = 0
    for bb in nc.main_func.blocks:
        il = bb.instructions
        i = 0
        while i < len(il):
            ins = il[i]
            lim = 0 if type(ins).__name__ == "InstDrain" else limit
            si = ins.sync_info
            waits = list(si.on_wait) if (si and si.on_wait) else []
            if type(ins).__name__ != "InstEventSemaphore" and len(waits) > lim:
                keep, extra = waits[:lim], waits[lim:]
                si.on_wait.clear()
                for w in keep:
                    si.on_wait.append(w)
                for j, w in enumerate(extra):
                    il.insert(i + j, _mk_wait(ins.engine, w))
                i += len(extra)
                n += 1
            i += 1
    return n


def _finalize(nc):
    from concourse.library_overlay import lower_extended_insts
    lower_extended_insts(nc)
    return _fix_waits(nc)


def _patched_drain_and_barrier(self, tick_clock, wait_clock):
    nc = self.nc
    probe = nc.sync.nop()
    wait_clock.add_sem_waits(probe.ins, ScopedClock({None: tick_clock.global_clock}))
    si = probe.ins.sync_info
    waits = list(si.on_wait or []) if si is not None else []
    if si is not None and si.on_wait:
        si.on_wait.clear()
    handles = list(self.sems.allocated().values())
    by_name = {getattr(h, "name", None): h for h in handles}
    for w in waits:
        nc.sync.wait_ge(by_name[w.ant_name], w.wait_value)
    nc.sync.drain()
    nc.all_engine_barrier()
    popped = nc._tile_sem_poison_stack.pop()
    assert popped is self._sem_poison
    nc.clear_and_free_semaphores(handles)
    nc.all_engine_barrier()


TileContext._drain_and_barrier = _patched_drain_and_barrier

# ----------------------------------------------------------------------------
# Kernel build
# ----------------------------------------------------------------------------
F32 = mybir.dt.float32
F32R = mybir.dt.float32r

NCORES = 8
B = 16384
BL = B // NCORES          # 2048 per core
IN = 784
H = 128
T = 16
FT = 7                    # feature tiles
F = FT * 128              # 896 padded
BC = 512                  # batch chunk (psum-bank limited)
NCH = BL // BC            # 4 chunks
NBT = BC // 128           # 4 batch subtiles per chunk

_CACHE = {}


def _build():
    if "nc" in _CACHE:
        return _CACHE["nc"]
    nc = bass.Bass(target_bir_lowering=False)
    x_ext = nc.declare_dram_parameter("x", [BL, IN], F32, isOutput=False)
    w_ext = nc.declare_dram_parameter("wts", [128, 4096], F32R, isOutput=False)
    sc_ext = nc.declare_dram_parameter("sc", [1, 1], F32, isOutput=False)
    o_ext = nc.declare_dram_parameter("out", [BL, IN], F32, isOutput=True)

    with TileContext(nc) as tc:
        with (tc.tile_pool(name="const", bufs=1) as constp,
              tc.tile_pool(name="sb", bufs=2) as sb,
              tc.tile_pool(name="st", bufs=1) as st,
              tc.tile_pool(name="st2", bufs=2) as st2,
              tc.tile_pool(name="ps", bufs=1, space="PSUM") as ps):

            wts = constp.tile([128, 4096], F32R, tag="wts")
            ident = constp.tile([128, 128], F32, tag="ident")
            scb = constp.tile([128, 1], F32, tag="scb")
            nc.sync.dma_start(out=wts[:], in_=w_ext[:])
            nc.sync.dma_start(out=scb[:], in_=sc_ext[:].to_broadcast([128, 1]))
            make_identity(nc, ident[:])
            nc.vector.tensor_scalar_mul(scb[:], scb[:], 1.0 / T)

            def _wslices(base):
                return ([wts[:, base + k * 128:base + (k + 1) * 128]
                         for k in range(FT)],
                        wts[:, base + 896:base + 1024],
                        wts[:, base + 1024:base + 1152],
                        [wts[:, base + 1152 + j * 128:base + 1152 + (j + 1) * 128]
                         for j in range(FT)])
            w1s, w2s, w3s, w4s = zip(_wslices(0), _wslices(2048))

            for c in range(NCH):
                inp = sb.tile([128, FT, BC], F32R, tag="inp", name=f"inp_{c}")
                Rb = [st.tile([128, FT, BC], F32R, tag=f"R{i}", name=f"R{i}_{c}")
                      for i in range(2)]
                g1 = st.tile([128, BC], F32, tag="g1", name=f"g1_{c}")
                h2 = st.tile([128, BC], F32R, tag="h2", name=f"h2_{c}")
                h3b = [st.tile([128, BC], F32R, tag=f"h3{i}", name=f"h3{i}_{c}")
                       for i in range(2)]
                A1 = st.tile([128, BC], F32, tag="A1", name=f"A1_{c}")
                C = st2.tile([128, FT, BC], F32, tag="C", name=f"C_{c}")

                # ---- input: DMA batch-major, PE-transpose, quantize ----
                for b in range(NBT):
                    xt = sb.tile([128, IN], F32, tag="x", name=f"x_{c}_{b}")
                    nc.sync.dma_start(
                        out=xt[:],
                        in_=x_ext[c * BC + b * 128:c * BC + (b + 1) * 128, :])
                    xps = ps.tile([128, 4, 128], F32, tag="xtps",
                                  name=f"xpsA_{c}_{b}")
                    for j in range(4):
                        nc.tensor.transpose(
                            xps[:, j, :], xt[:, j * 128:(j + 1) * 128],
                            identity=ident[:])
                    nc.vector._custom_dve(
                        ANT_ROUND16,
                        out=inp[:, 0:4, b * 128:(b + 1) * 128],
                        in0=xps[:, :, :],
                        s0=16.0, s1=float(2 ** 23), imm2=1.0 / 16.0)
                    xps2 = ps.tile([128, 3, 128], F32, tag="xtps",
                                   name=f"xpsB_{c}_{b}")
                    for j in range(4, 6):
                        nc.tensor.transpose(
                            xps2[:, j - 4, :], xt[:, j * 128:(j + 1) * 128],
                            identity=ident[:])
                    nc.vector.memset(xps2[:, 2, :], 0.0)
                    nc.tensor.transpose(
                        xps2[0:16, 2, :], xt[:, 768:784], identity=ident[:])
                    nc.vector._custom_dve(
                        ANT_ROUND16,
                        out=inp[:, 4:7, b * 128:(b + 1) * 128],
                        in0=xps2[:, :, :],
                        s0=16.0, s1=float(2 ** 23), imm2=1.0 / 16.0)

                nc.gpsimd.memset(g1[:], 0.0)
                nc.gpsimd.memset(C[:], 0.0)

                # ---- A1 = W1 @ inp (once per chunk) ----
                a1ps = ps.tile([128, BC], F32, tag="xtps", name=f"a1ps_{c}")
                for k in range(FT):
                    for h in range(2):
                        nc.tensor.matmul(a1ps[:], w1s[h][k], inp[:, k, :],
                                         start=(k == 0 and h == 0),
                                         stop=(k == FT - 1 and h == 1))
                # stationary is -W1, so negate to get +A1
                nc.vector.tensor_scalar_mul(A1[:], a1ps[:], -1.0)

                V123 = ps.tile([128, 3, BC], F32, tag="V123", name=f"V123_{c}")
                V1 = V123[:, 0, :]
                V2 = V123[:, 1, :]
                V3 = V123[:, 2, :]

                # ---- 16 steps; layer-4 psum is FRESH per (step, half):
                #      V4 = -W4 @ h3_t  (h3 = -cum spikes, exact), so no s3
                #      materialization and only one 4-bank psum slot for D ----
                HB = BC // 2
                # prologue: R_1 = fire(0 + inp)
                nc.vector._custom_dve(ANT_FIRE0, out=Rb[0][:, 0:4, :], in0=inp[:, 0:4, :])
                nc.vector._custom_dve(ANT_FIRE0, out=Rb[0][:, 4:7, :], in0=inp[:, 4:7, :])

                def d_block(tprev, halves=(0, 1)):
                    """layer 4 for step tprev (runs one iteration later):
                    V4 = -W4 @ h3_tprev, fresh psum per half + COUNT."""
                    h3 = h3b[tprev % 2]
                    for half in halves:
                        lo = half * HB
                        V4h = ps.tile([128, FT, HB], F32, tag="V4h",
                                      name=f"V4h_{c}_{tprev}_{half}")
                        for j in range(FT):
                            for h in range(2):
                                nc.tensor.matmul(
                                    V4h[:, j, :], w4s[h][j], h3[:, lo:lo + HB],
                                    start=(h == 0), stop=(h == 1))
                        nc.vector._custom_dve(
                            ANT_COUNT, out=C[:, :, lo:lo + HB],
                            in0=C[:, :, lo:lo + HB], in1=V4h[:])

                for t in range(T):
                    R = Rb[t % 2]
                    Rn = Rb[(t + 1) % 2]
                    s1 = sb.tile([128, BC], F32R, tag="s1", name=f"s1_{c}_{t}")
                    # V1_t = t*A1 - W1 @ R_t   (fresh psum each step)
                    for k in range(FT):
                        for h in range(2):
                            nc.tensor.matmul(V1[:], w1s[h][k], R[:, k, :],
                                             start=(k == 0 and h == 0),
                                             stop=(k == FT - 1 and h == 1))
                    # layer-4 work of the PREVIOUS step fills the L2/L3 stretch
                    if t > 0:
                        d_block(t - 1, halves=(0,))
                    if t < T - 1:
                        nc.vector._custom_dve(ANT_FIRE, out=Rn[:, 0:4, :],
                                              in0=R[:, 0:4, :], in1=inp[:, 0:4, :])
                    nc.gpsimd.tensor_add(g1[:], g1[:], A1[:])
                    nc.vector._custom_dve(ANT_SPIKE, out=s1[:], in0=g1[:], in1=V1[:])
                    if t < T - 1:
                        nc.vector._custom_dve(ANT_FIRE, out=Rn[:, 4:7, :],
                                              in0=R[:, 4:7, :], in1=inp[:, 4:7, :])
                    nc.vector._custom_dve(ANT_DROP, out=g1[:], in0=g1[:], in1=V1[:])
                    for h in range(2):
                        nc.tensor.matmul(V2[:], w2s[h], s1[:],
                                         start=(t == 0 and h == 0),
                                         stop=(t == T - 1 and h == 1))
                    if t == 0:
                        nc.vector._custom_dve(ANT_DROP0, out=h2[:], in0=V2[:])
                    else:
                        nc.vector._custom_dve(ANT_DROP, out=h2[:], in0=h2[:], in1=V2[:])
                    # V3_t = -W3 @ h2_t  (h2 = -cum spikes, exact; fresh psum)
                    for h in range(2):
                        nc.tensor.matmul(V3[:], w3s[h], h2[:],
                                         start=(h == 0), stop=(h == 1))
                    if t > 0:
                        d_block(t - 1, halves=(1,))
                    if t == 0:
                        nc.vector._custom_dve(ANT_DROP0, out=h3b[0][:], in0=V3[:])
                    else:
                        nc.vector._custom_dve(ANT_DROP, out=h3b[t % 2][:],
                                              in0=h3b[(t - 1) % 2][:], in1=V3[:])
                # epilogue: layer 4 of the final step
                d_block(T - 1)

                # ---- output: transpose back, scale, DMA ----
                for b in range(NBT):
                    cps = ps.tile([128, FT, 128], F32, tag="V4h",
                                  name=f"coT_{c}_{b}")
                    for j in range(FT):
                        nc.tensor.transpose(
                            cps[:, j, :], C[:, j, b * 128:(b + 1) * 128],
                            identity=ident[:])
                    yo = sb.tile([128, FT, 128], F32, tag="yo",
                                 name=f"yo_{c}_{b}")
                    nc.vector.tensor_scalar(
                        out=yo[:], in0=cps[:], scalar1=scb[:], scalar2=None,
                        op0=mybir.AluOpType.mult)
                    orows = slice(c * BC + b * 128, c * BC + (b + 1) * 128)
                    for j in range(6):
                        nc.gpsimd.dma_start(
                            out=o_ext[orows, j * 128:(j + 1) * 128],
                            in_=yo[:, j, :])
                    nc.gpsimd.dma_start(out=o_ext[orows, 768:784],
                                        in_=yo[:, 6, 0:16])

    _finalize(nc)
    _CACHE["nc"] = nc
    return nc


def _rne11(x):
    xi = np.asarray(x, np.float32).view(np.uint32).astype(np.uint64)
    half = np.uint64(1 << 11)
    lsb = (xi >> np.uint64(12)) & np.uint64(1)
    q = ((xi + half - np.uint64(1) + lsb) >> np.uint64(12)) << np.uint64(12)
    return np.minimum(q, np.uint64(0xFFFFFFFF)).astype(np.uint32).view(np.float32)


def _prep_inputs(features, W1, W2, W3, W4, out_scale):
    f32 = np.float32
    W1p = np.zeros((H, F), f32); W1p[:, :IN] = W1
    W4p = np.zeros((F, H), f32); W4p[:IN, :] = W4
    W1T = W1p.T.reshape(FT, 128, H).transpose(1, 0, 2).reshape(128, FT * H)
    whole = np.concatenate(
        [-W1T, W2.T.astype(f32), -W3.T.astype(f32), -W4p.T], axis=1)
    hi = _rne11(whole)
    lo = (whole - hi).astype(f32)
    wts = np.ascontiguousarray(np.concatenate([hi, lo], axis=1), dtype=f32)
    sc = np.asarray(out_scale, f32).reshape(1, 1)
    in_maps = []
    for i in range(NCORES):
        in_maps.append({
            "x": np.ascontiguousarray(features[i * BL:(i + 1) * BL], f32),
            "wts": wts,
            "sc": sc,
        })
    return in_maps


def _run(inputs, trace=False):
    nc = _build()
    in_maps = _prep_inputs(**inputs)
    res = run_bass_kernel_spmd(nc, in_maps, core_ids=list(range(NCORES)),
                               trace=trace)
    out = np.concatenate([res.results[i]["out"] for i in range(NCORES)], axis=0)
    return out.astype(np.float32), res


def kernel(**inputs):
    out, _ = _run(inputs, trace=False)
    return out


# revision 4
# speedup vs baseline: 1.4001x; 1.0119x over previous
"""Spiking autoencoder (integrate-and-fire, 16 timesteps) on 8 TRN2 NeuronCores.

Data-parallel: batch 16384 split as 8 x 2048. Per core, a fully fused
Bass/Tile kernel:

  - features are PE-transposed to feature-major [896(pad), B] layout and
    quantized (round(16x)/16) with one fused DVE op.
  - Each layer's membrane potential uses the cumulative-potential trick:
    PSUM accumulates V_t = sum_{tau<=t} W @ s_tau across all 16 steps
    (matmul start= only on t==0), and the integrate-and-fire recurrence is
    run against V_t with an integer spike-count state:
        s_t = [h + V_t >= 1],  h -= s_t        (h = -#spikes so far, exact)
    which equals the reference's (p += u; s = [p>=1]; p -= s) chain.
  - Layers 3/4 exploit that the h-state is an exact integer count: their
    potentials are rebuilt fresh each step as -W @ h (so s2/s3 are never
    materialized), and layer 4 keeps only the output spike COUNT:
    C += [V4 - C >= 1].
  - Matmuls run as float32r (full-rate on the PE array); every weight is
    split hi=rne11(W), lo=W-hi so two accumulating fp32r matmuls reproduce
    fp32 precision.

Custom DVE ops (fused, registered at import): ANT_SPIKE, ANT_FIRE,
ANT_DROP, ANT_COUNT, ANT_ROUND16.
"""
import sys
import copy
import itertools

sys.path.insert(0, "/opt/trn_rl_repo")

import numpy as np

# ----------------------------------------------------------------------------
# Custom DVE op registration
# ----------------------------------------------------------------------------
import concourse.dve_ops as dve_ops
from concourse.dve_ops import DveOp
from concourse.dve_spec import (
    Spec, Src0, Src1, One, Zero, C0, C1, C2, lower, _has_src1 as has_src1,
)
from concourse.dve_table_gen import DveOpSpec

_F = np.float32


def _register(name, spec):
    if name in dve_ops._SUB_OPCODE_FOR_NAME:
        return next(op for op in dve_ops.OPS if op.name == name)
    shas = {}
    for ver in ("v3", "v4"):
        s = DveOpSpec(name=name, opcode=0, uops=lower(spec, ver=ver),
                      rd1_en=has_src1(spec))
        shas[ver] = s.sha(ver)
    op = DveOp(name, spec, subdim=False, uops_sha=shas)
    dve_ops.OPS.append(op)
    dve_ops._SUB_OPCODE_FOR_NAME[name] = (
        dve_ops._CUSTOM_DVE_ROW_BASE + len(dve_ops.OPS) - 1)
    dve_ops.CUSTOM_DVE_SPECS[name] = spec
    assert dve_ops._SUB_OPCODE_FOR_NAME[name] < 0x20
    return op


# s = (p + u) >= 1
ANT_SPIKE = _register("ANT_SPIKE", Spec(
    body=(Src0 + Src1) >= One,
    reference=lambda in0, in1, s0, s1, imm2: ((in0 + in1) >= 1.0).astype(_F)))

# p' = q - (q >= 1), q = p + u   (state update when u is the per-step input)
_q = Src0 + Src1
ANT_FIRE = _register("ANT_FIRE", Spec(
    body=_q - (_q >= One),
    reference=lambda in0, in1, s0, s1, imm2:
        (in0 + in1) - ((in0 + in1) >= 1.0).astype(_F)))

# h' = h - ((h + V) >= 1)   (state update when V is cumulative)
ANT_DROP = _register("ANT_DROP", Spec(
    body=Src0 - ((Src0 + Src1) >= One),
    reference=lambda in0, in1, s0, s1, imm2:
        in0 - ((in0 + in1) >= 1.0).astype(_F)))

# C' = C + ((V - C) >= 1)
ANT_COUNT = _register("ANT_COUNT", Spec(
    body=Src0 + ((Src1 - Src0) >= One),
    reference=lambda in0, in1, s0, s1, imm2:
        in0 + ((in1 - in0) >= 1.0).astype(_F)))

# R_1 = x - (x >= 1)   (first fire step from zero state)
ANT_FIRE0 = _register("ANT_FIRE0", Spec(
    body=Src0 - (Src0 >= One),
    reference=lambda in0, s0, s1, imm2: in0 - (in0 >= 1.0).astype(_F)))

# h' = -(V >= 1)   (first cumulative-drop step from zero state)
ANT_DROP0 = _register("ANT_DROP0", Spec(
    body=Zero - (Src0 >= One),
    reference=lambda in0, s0, s1, imm2: -((in0 >= 1.0).astype(_F))))

# F' = F + [(inp*t - F) >= 1]   (layer-0 cumulative spike count, one op)
ANT_FCOUNT = _register("ANT_FCOUNT", Spec(
    body=Src0 + ((Src1 * C0 - Src0) >= One),
    reference=lambda in0, in1, s0, s1, imm2:
        in0 + ((in1 * s0 - in0) >= 1.0).astype(_F)))

# F_1 = [x >= 1]   (first count step from zero state)
ANT_GE1 = _register("ANT_GE1", Spec(
    body=(Src0 >= One) + Zero,
    reference=lambda in0, s0, s1, imm2: (in0 >= 1.0).astype(_F)))

# out = ((x*C0 + C1) - C1) * C2  -> round-to-nearest-even via the 2^23 trick
ANT_ROUND16 = _register("ANT_ROUND16", Spec(
    body=((Src0 * C0 + C1) - C1) * C2,
    reference=lambda in0, s0, s1, imm2: (
        (np.float32(in0 * np.float32(s0)) + np.float32(s1))
        - np.float32(s1)) * np.float32(imm2)))

# ----------------------------------------------------------------------------
# Walrus-compat fixes (this container's neuronxcc rejects >1 sem-wait on
# many instruction structs and any wait on InstDrain; raw Bass also skips
# the pass that packs extended-inst ISA bytes).
# ----------------------------------------------------------------------------
from concourse import bass, mybir
from concourse.tile import TileContext
from concourse.vector_clock import ScopedClock
from concourse.bass_utils import run_bass_kernel_spmd
from concourse.masks import make_identity

_ctr = itertools.count()


def _build_wait_templates():
    nc = bass.Bass(target_bir_lowering=False)
    out = {}
    with nc.Block() as block, nc.semaphore("s") as s:
        for eng_name in ("sync", "vector", "scalar", "gpsimd", "tensor"):
            def _mk(e, _out=out):
                i = e.wait_ge(s, 0)
                _out[i.ins.engine] = i.ins
            getattr(block, eng_name)(_mk)
    return out


_WAIT_TEMPLATES = _build_wait_templates()


def _mk_wait(engine, w):
    wi = copy.deepcopy(_WAIT_TEMPLATES[engine])
    wi.name = f"I-waitsplit-{next(_ctr)}"
    si = wi.sync_info
    si.on_wait.clear()
    si.on_wait.append(w)
    return wi


def _fix_waits(nc, limit=1):
    n = 0
    for bb in nc.main_func.blocks:
        il = bb.instructions
        i = 0
        while i < len(il):
            ins = il[i]
            lim = 0 if type(ins).__name__ == "InstDrain" else limit
            si = ins.sync_info
            waits = list(si.on_wait) if (si and si.on_wait) else []
            if type(ins).__name__ != "InstEventSemaphore" and len(waits) > lim:
                keep, extra = waits[:lim], waits[lim:]
                si.on_wait.clear()
                for w in keep:
                    si.on_wait.append(w)
                for j, w in enumerate(extra):
                    il.insert(i + j, _mk_wait(ins.engine, w))
                i += len(extra)
                n += 1
            i += 1
    return n


def _finalize(nc):
    from concourse.library_overlay import lower_extended_insts
    lower_extended_insts(nc)
    return _fix_waits(nc)


def _patched_drain_and_barrier(self, tick_clock, wait_clock):
    nc = self.nc
    probe = nc.sync.nop()
    wait_clock.add_sem_waits(probe.ins, ScopedClock({None: tick_clock.global_clock}))
    si = probe.ins.sync_info
    waits = list(si.on_wait or []) if si is not None else []
    if si is not None and si.on_wait:
        si.on_wait.clear()
    handles = list(self.sems.allocated().values())
    by_name = {getattr(h, "name", None): h for h in handles}
    for w in waits:
        nc.sync.wait_ge(by_name[w.ant_name], w.wait_value)
    nc.sync.drain()
    nc.all_engine_barrier()
    popped = nc._tile_sem_poison_stack.pop()
    assert popped is self._sem_poison
    nc.clear_and_free_semaphores(handles)
    nc.all_engine_barrier()


TileContext._drain_and_barrier = _patched_drain_and_barrier

# ----------------------------------------------------------------------------
# Kernel build
# ----------------------------------------------------------------------------
F32 = mybir.dt.float32
F32R = mybir.dt.float32r

NCORES = 8
B = 16384
BL = B // NCORES          # 2048 per core
IN = 784
H = 128
T = 16
FT = 7                    # feature tiles
F = FT * 128              # 896 padded
BC = 512                  # batch chunk (psum-bank limited)
NCH = BL // BC            # 4 chunks
NBT = BC // 128           # 4 batch subtiles per chunk

_CACHE = {}


def _build():
    if "nc" in _CACHE:
        return _CACHE["nc"]
    nc = bass.Bass(target_bir_lowering=False)
    x_ext = nc.declare_dram_parameter("x", [BL, IN], F32, isOutput=False)
    w_ext = nc.declare_dram_parameter("wts", [128, 4096], F32R, isOutput=False)
    sc_ext = nc.declare_dram_parameter("sc", [1, 1], F32, isOutput=False)
    o_ext = nc.declare_dram_parameter("out", [BL, IN], F32, isOutput=True)

    with TileContext(nc) as tc:
        with (tc.tile_pool(name="const", bufs=1) as constp,
              tc.tile_pool(name="sb", bufs=2) as sb,
              tc.tile_pool(name="st", bufs=1) as st,
              tc.tile_pool(name="st2", bufs=2) as st2,
              tc.tile_pool(name="ps", bufs=1, space="PSUM") as ps):

            wts = constp.tile([128, 4096], F32R, tag="wts")
            ident = constp.tile([128, 128], F32, tag="ident")
            scb = constp.tile([128, 1], F32, tag="scb")
            nc.sync.dma_start(out=wts[:], in_=w_ext[:])
            nc.sync.dma_start(out=scb[:], in_=sc_ext[:].to_broadcast([128, 1]))
            make_identity(nc, ident[:])
            nc.vector.tensor_scalar_mul(scb[:], scb[:], 1.0 / T)

            def _wslices(base):
                return ([wts[:, base + k * 128:base + (k + 1) * 128]
                         for k in range(FT)],
                        wts[:, base + 896:base + 1024],
                        wts[:, base + 1024:base + 1152],
                        [wts[:, base + 1152 + j * 128:base + 1152 + (j + 1) * 128]
                         for j in range(FT)])
            w1s, w2s, w3s, w4s = zip(_wslices(0), _wslices(2048))

            for c in range(NCH):
                inp = sb.tile([128, FT, BC], F32R, tag="inp", name=f"inp_{c}")
                Rb = [st.tile([128, FT, BC], F32R, tag=f"R{i}", name=f"R{i}_{c}")
                      for i in range(2)]
                h1 = st.tile([128, BC], F32R, tag="h1", name=f"h1_{c}")
                h2 = st.tile([128, BC], F32R, tag="h2", name=f"h2_{c}")
                h3b = [st.tile([128, BC], F32R, tag=f"h3{i}", name=f"h3{i}_{c}")
                       for i in range(2)]
                C = st2.tile([128, FT, BC], F32, tag="C", name=f"C_{c}")

                # ---- input: DMA batch-major, PE-transpose, quantize ----
                for b in range(NBT):
                    xt = sb.tile([128, IN], F32, tag="x", name=f"x_{c}_{b}")
                    nc.sync.dma_start(
                        out=xt[:],
                        in_=x_ext[c * BC + b * 128:c * BC + (b + 1) * 128, :])
                    xps = ps.tile([128, 4, 128], F32, tag="xtps",
                                  name=f"xpsA_{c}_{b}")
                    for j in range(4):
                        nc.tensor.transpose(
                            xps[:, j, :], xt[:, j * 128:(j + 1) * 128],
                            identity=ident[:])
                    nc.vector._custom_dve(
                        ANT_ROUND16,
                        out=inp[:, 0:4, b * 128:(b + 1) * 128],
                        in0=xps[:, :, :],
                        s0=16.0, s1=float(2 ** 23), imm2=1.0 / 16.0)
                    xps2 = ps.tile([128, 3, 128], F32, tag="xtps",
                                   name=f"xpsB_{c}_{b}")
                    for j in range(4, 6):
                        nc.tensor.transpose(
                            xps2[:, j - 4, :], xt[:, j * 128:(j + 1) * 128],
                            identity=ident[:])
                    nc.vector.memset(xps2[:, 2, :], 0.0)
                    nc.tensor.transpose(
                        xps2[0:16, 2, :], xt[:, 768:784], identity=ident[:])
                    nc.vector._custom_dve(
                        ANT_ROUND16,
                        out=inp[:, 4:7, b * 128:(b + 1) * 128],
                        in0=xps2[:, :, :],
                        s0=16.0, s1=float(2 ** 23), imm2=1.0 / 16.0)

                nc.gpsimd.memset(C[:], 0.0)

                V123 = ps.tile([128, 3, BC], F32, tag="V123", name=f"V123_{c}")
                V1 = V123[:, 0, :]
                V2 = V123[:, 1, :]
                V3 = V123[:, 2, :]

                # ---- 16 steps; layer-4 psum is FRESH per (step, half):
                #      V4 = -W4 @ h3_t  (h3 = -cum spikes, exact), so no s3
                #      materialization and only one 4-bank psum slot for D ----
                HB = BC // 2
                # prologue: F_1 = [inp >= 1]
                nc.vector._custom_dve(ANT_GE1, out=Rb[0][:, 0:4, :], in0=inp[:, 0:4, :])
                nc.vector._custom_dve(ANT_GE1, out=Rb[0][:, 4:7, :], in0=inp[:, 4:7, :])

                def d_block(tprev, halves=(0, 1)):
                    """layer 4 for step tprev (runs one iteration later):
                    V4 = -W4 @ h3_tprev, fresh psum per half + COUNT."""
                    h3 = h3b[tprev % 2]
                    for half in halves:
                        lo = half * HB
                        V4h = ps.tile([128, FT, HB], F32, tag="V4h",
                                      name=f"V4h_{c}_{tprev}_{half}")
                        for j in range(FT):
                            for h in range(2):
                                nc.tensor.matmul(
                                    V4h[:, j, :], w4s[h][j], h3[:, lo:lo + HB],
                                    start=(h == 0), stop=(h == 1))
                        nc.vector._custom_dve(
                            ANT_COUNT, out=C[:, :, lo:lo + HB],
                            in0=C[:, :, lo:lo + HB], in1=V4h[:])

                for t in range(T):
                    F = Rb[t % 2]
                    Fn = Rb[(t + 1) % 2]
                    # V1_t = W1 @ F_t   (F = cum layer-0 spike count; fresh psum)
                    for k in range(FT):
                        for h in range(2):
                            nc.tensor.matmul(V1[:], w1s[h][k], F[:, k, :],
                                             start=(k == 0 and h == 0),
                                             stop=(k == FT - 1 and h == 1))
                    # layer-4 work of the PREVIOUS step fills the L2/L3 stretch
                    if t > 0:
                        d_block(t - 1, halves=(0,))
                    if t < T - 1:
                        nc.vector._custom_dve(ANT_FCOUNT, out=Fn[:, 0:4, :],
                                              in0=F[:, 0:4, :], in1=inp[:, 0:4, :],
                                              s0=float(t + 2))
                        nc.vector._custom_dve(ANT_FCOUNT, out=Fn[:, 4:7, :],
                                              in0=F[:, 4:7, :], in1=inp[:, 4:7, :],
                                              s0=float(t + 2))
                    if t == 0:
                        nc.vector._custom_dve(ANT_DROP0, out=h1[:], in0=V1[:])
                    else:
                        nc.vector._custom_dve(ANT_DROP, out=h1[:], in0=h1[:], in1=V1[:])
                    # V2_t = -W2 @ h1_t  (h1 = -cum spikes, exact; fresh psum)
                    for h in range(2):
                        nc.tensor.matmul(V2[:], w2s[h], h1[:],
                                         start=(h == 0), stop=(h == 1))
                    if t == 0:
                        nc.vector._custom_dve(ANT_DROP0, out=h2[:], in0=V2[:])
                    else:
                        nc.vector._custom_dve(ANT_DROP, out=h2[:], in0=h2[:], in1=V2[:])
                    # V3_t = -W3 @ h2_t  (h2 = -cum spikes, exact; fresh psum)
                    for h in range(2):
                        nc.tensor.matmul(V3[:], w3s[h], h2[:],
                                         start=(h == 0), stop=(h == 1))
                    if t > 0:
                        d_block(t - 1, halves=(1,))
                    if t == 0:
                        nc.vector._custom_dve(ANT_DROP0, out=h3b[0][:], in0=V3[:])
                    else:
                        nc.vector._custom_dve(ANT_DROP, out=h3b[t % 2][:],
                                              in0=h3b[(t - 1) % 2][:], in1=V3[:])
                # epilogue: layer 4 of the final step
                d_block(T - 1)

                # ---- output: transpose back, scale, DMA ----
                for b in range(NBT):
                    cps = ps.tile([128, FT, 128], F32, tag="V4h",
                                  name=f"coT_{c}_{b}")
                    for j in range(FT):
                        nc.tensor.transpose(
                            cps[:, j, :], C[:, j, b * 128:(b + 1) * 128],
                            identity=ident[:])
                    yo = sb.tile([128, FT, 128], F32, tag="yo",
                                 name=f"yo_{c}_{b}")
                    nc.vector.tensor_scalar(
                        out=yo[:], in0=cps[:], scalar1=scb[:], scalar2=None,
                        op0=mybir.AluOpType.mult)
                    orows = slice(c * BC + b * 128, c * BC + (b + 1) * 128)
                    for j in range(6):
                        nc.gpsimd.dma_start(
                            out=o_ext[orows, j * 128:(j + 1) * 128],
                            in_=yo[:, j, :])
                    nc.gpsimd.dma_start(out=o_ext[orows, 768:784],
                                        in_=yo[:, 6, 0:16])

    _finalize(nc)
    _CACHE["nc"] = nc
    return nc


def _rne11(x):
    xi = np.asarray(x, np.float32).view(np.uint32).astype(np.uint64)
    half = np.uint64(1 << 11)
    lsb = (xi >> np.uint64(12)) & np.uint64(1)
    q = ((xi + half - np.uint64(1) + lsb) >> np.uint64(12)) << np.uint64(12)
    return np.minimum(q, np.uint64(0xFFFFFFFF)).astype(np.uint32).view(np.float32)


def _prep_inputs(features, W1, W2, W3, W4, out_scale):
    f32 = np.float32
    W1p = np.zeros((H, F), f32); W1p[:, :IN] = W1
    W4p = np.zeros((F, H), f32); W4p[:IN, :] = W4
    W1T = W1p.T.reshape(FT, 128, H).transpose(1, 0, 2).reshape(128, FT * H)
    whole = np.concatenate(
        [W1T, -W2.T.astype(f32), -W3.T.astype(f32), -W4p.T], axis=1)
    hi = _rne11(whole)
    lo = (whole - hi).astype(f32)
    wts = np.ascontiguousarray(np.concatenate([hi, lo], axis=1), dtype=f32)
    sc = np.asarray(out_scale, f32).reshape(1, 1)
    in_maps = []
    for i in range(NCORES):
        in_maps.append({
            "x": np.ascontiguousarray(features[i * BL:(i + 1) * BL], f32),
            "wts": wts,
            "sc": sc,
        })
    return in_maps


def _run(inputs, trace=False):
    nc = _build()
    in_maps = _prep_inputs(**inputs)
    res = run_bass_kernel_spmd(nc, in_maps, core_ids=list(range(NCORES)),
                               trace=trace)
    out = np.concatenate([res.results[i]["out"] for i in range(NCORES)], axis=0)
    return out.astype(np.float32), res


def kernel(**inputs):
    out, _ = _run(inputs, trace=False)
    return out
